# revision 20
# baseline (speedup 1.0000x reference)
"""Chamfer one-direction nearest-neighbor distance on 8 Trainium2 NeuronCores.

For each of 65536 query points (pc0) find min squared distance to 65536
points (pc1), then return mean over queries with min-dist <= 2.0.

Strategy (v2):
  - Inputs shipped as fp16 (halves wire traffic over the axon tunnel;
    adds ~5e-5 relative error vs the 2e-2 gate). BOTH pc0 and pc1 are
    sharded 8-way; pc1 is replicated on-device via a DRAM AllGather
    collective, so each f32 input crosses the tunnel exactly once.
  - Distance matrix tile via K=7 fp16 matmul using the augmentation
      d = |a|^2 + |b|^2 - 2 a.b
    with A' = [a, n2a_hi, n2a_lo, 1, 1] and B' = [-2b, 1, 1, n2b_hi,
    n2b_lo]; all products of fp16 pairs are exact in fp32.
  - Transposed operands built with DVE StreamTranspose of point-major
    32-wide feature tiles (no strided DMA gathers, no DRAM scratch).
    The resulting band/column permutations are harmless: min over
    candidates and masked-sum over queries are order-invariant.
  - Min-reduction of PSUM tiles: Act copies every other tile to SBUF,
    DVE tensor_tensor_scan(min) consumes (PSUM, SBUF) pairs in two
    interleaved chains -- the max PSUM egress TRN2 allows (GPSIMD has
    no PSUM access; InstPool is DVE-only).
  - Per-core [128,2] masked-sum/count is AllReduced on device, so the
    host fetches one replicated [128,2] buffer from a single device.
  - Host caches the compiled jit; per call only input transfer +
    execute + tiny fetch.
"""

import os

os.environ.setdefault("NEURON_RT_RESET_CORES", "1")
os.environ.setdefault("JAX_PLATFORMS", "axon,cpu")

import numpy as np

try:  # reuse compiled NEFF across processes when possible
    import jax

    jax.config.update("jax_compilation_cache_dir", "/tmp/jaxcache")
    jax.config.update("jax_persistent_cache_min_entry_size_bytes", -1)
    jax.config.update("jax_persistent_cache_min_compile_time_secs", 0)
except Exception:
    pass

N_CORES = 8
NQ = 65536 // N_CORES  # queries per core
NP = 65536             # candidate points (replicated on-device)
BIG = 1.0e30

_STATE = {}


def build_nc(spmd=True):
    """Per-core Bass program. spmd=False builds a single-core variant
    (full pc1 as direct input, no collective) for simulation."""
    import concourse.bacc as bacc
    import concourse.tile as tile
    from concourse import mybir

    f32, f16 = mybir.dt.float32, mybir.dt.float16
    AX, OP = mybir.AxisListType, mybir.AluOpType

    nb = NP // 128          # 512 candidate rows per partition
    na = NQ // 32           # 256 query rows per partition band
    iblocks = NQ // 128     # 64
    chunks = (NP // 4) // 512  # 32 moving chunks per band
    ntiles = 4 * chunks // 2   # 64 psum tiles [128,1024] per iblock
    nshard = NP // N_CORES

    nc = bacc.Bacc("TRN2", target_bir_lowering=False,
                   num_devices=N_CORES if spmd else None)
    if spmd:
        # one combined input: rows [0,NQ) = pc0 shard, [NQ,NQ+nshard) = pc1
        # shard (fewer per-device transfer ops over the axon tunnel)
        pts = nc.dram_tensor("pts", [NQ + nshard, 3], f16, kind="ExternalInput")
        pc0h = pts[0:NQ, :]
        pc1h = pts[NQ:NQ + nshard, :]
        stage = nc.dram_tensor("stage", [nshard, 3], f16)
        pc1full = nc.dram_tensor("pc1full", [NP, 3], f16)
        pc1full_read = pc1full[:, :]
    else:
        pts = nc.dram_tensor("pts", [NQ + NP, 3], f16, kind="ExternalInput")
        pc0h = pts[0:NQ, :]
        pc1full_read = pts[NQ:NQ + NP, :]
    out = nc.dram_tensor("out", [128, 2], f32, kind="ExternalOutput")

    with tile.TileContext(nc) as tc:
        with tc.tile_pool(name="keep", bufs=1) as keep:
            if spmd:
                with tc.tile_pool(name="cc", bufs=1) as ccp:
                    s1 = ccp.tile([128, nshard // 128, 3], f16)
                    nc.sync.dma_start(
                        out=s1, in_=pc1h.rearrange("(p n) c -> p n c", p=128))
                    nc.sync.dma_start(
                        out=stage[:, :].rearrange("(p n) c -> p n c", p=128), in_=s1)
                    nc.gpsimd.collective_compute(
                        kind="AllGather",
                        op=OP.bypass,
                        replica_groups=[list(range(N_CORES))],
                        ins=[stage[:, :]],
                        outs=[pc1full[:, :]],
                    )

            rhs = keep.tile([128, nb, 32], f16)
            lhsT = keep.tile([128, na, 32], f16)

            def feats_common(prep, raw, n):
                rf = prep.tile([128, n, 3], f32, tag="rf")
                nc.vector.tensor_copy(rf, raw)
                sq = prep.tile([128, n, 3], f32, tag="sq")
                nc.vector.tensor_mul(sq, rf, rf)
                n2 = prep.tile([128, n], f32, tag="n2")
                nc.vector.tensor_reduce(out=n2, in_=sq, axis=AX.X, op=OP.add)
                n2h = prep.tile([128, n], f16, tag="n2h")
                nc.vector.tensor_copy(n2h, n2)
                n2l = prep.tile([128, n], f32, tag="n2l")
                nc.vector.tensor_sub(n2l, n2, n2h)
                return n2h, n2l

            with tc.tile_pool(name="prep", bufs=1) as prep:
                # ---- B side: candidates, point-major [p, i] = row p*nb+i
                raw_b = prep.tile([128, nb, 3], f16, tag="rawb")
                nc.sync.dma_start(
                    out=raw_b, in_=pc1full_read.rearrange("(p n) c -> p n c", p=128))
                bn2h, bn2l = feats_common(prep, raw_b, nb)
                fb = prep.tile([128, nb, 32], f16, tag="fb")
                nc.vector.memset(fb, 0.0)
                nc.scalar.mul(fb[:, :, 0:3], raw_b, -2.0)
                nc.vector.memset(fb[:, :, 3:5], 1.0)
                nc.vector.tensor_copy(fb[:, :, 5:6], bn2h[:, :, None])
                nc.vector.tensor_copy(fb[:, :, 6:7], bn2l[:, :, None])

                # ---- A side: queries, replicated into all 4 partition bands
                raw_a = prep.tile([128, na, 3], f16, tag="rawa")
                for d in range(4):
                    nc.sync.dma_start(
                        out=raw_a[32 * d: 32 * (d + 1), :, :],
                        in_=pc0h.rearrange("(p n) c -> p n c", p=32))
                an2h, an2l = feats_common(prep, raw_a, na)
                fa = prep.tile([128, na, 32], f16, tag="fa")
                nc.vector.memset(fa, 0.0)
                nc.vector.tensor_copy(fa[:, :, 0:3], raw_a)
                nc.vector.tensor_copy(fa[:, :, 3:4], an2h[:, :, None])
                nc.vector.tensor_copy(fa[:, :, 4:5], an2l[:, :, None])
                nc.vector.memset(fa[:, :, 5:7], 1.0)

                nc.vector.transpose(rhs, fb)
                nc.vector.transpose(lhsT, fa)

            # ---------------- main loop ------------------------------------
            # Per 128-query block: 64 PSUM tiles [128,1024]. On TRN2 only DVE
            # and Act touch PSUM (GPSIMD cannot; InstPool is DVE-only), so:
            # Act copies every other tile to SBUF, DVE tensor_tensor_scan(min)
            # consumes (PSUM, SBUF) pairs -- 2 tiles per 1024-cycle DVE op,
            # the max PSUM egress this architecture allows.
            # Engine busy/iblock: DVE 36.2us > Act 30.1 > PE 27.3.
            mins = keep.tile([128, iblocks], f32)
            with tc.tile_pool(name="psum", bufs=4, space="PSUM") as pp, \
                 tc.tile_pool(name="cpy", bufs=4) as cpp, \
                 tc.tile_pool(name="scn", bufs=4) as scn:
                for i in range(iblocks):
                    t = 0

                    def mm_tile():
                        nonlocal t
                        b, cpair = divmod(t, chunks // 2)
                        ps = pp.tile([128, 1024], f32, tag="ps")
                        for h in range(2):
                            ci = 2 * cpair + h
                            nc.tensor.matmul(
                                ps[:, h * 512:(h + 1) * 512],
                                lhsT[32 * b: 32 * b + 7, 4 * i: 4 * i + 4, :],
                                rhs[32 * b: 32 * b + 7, 16 * ci: 16 * (ci + 1), :],
                                start=True, stop=True,
                                tile_position=(32 * b, 0),
                            )
                        t += 1
                        return ps

                    # two interleaved scan chains: each scan depends on the
                    # scan two steps back, hiding the Act-copy latency
                    prev = [None, None]
                    for s in range(ntiles // 2):
                        psB = mm_tile()
                        sbB = cpp.tile([128, 1024], f32, tag="sbB")
                        nc.scalar.copy(sbB, psB)
                        psA = mm_tile()
                        tr = scn.tile([128, 1024], f32, tag="tr")
                        c = s & 1
                        init = BIG if prev[c] is None else prev[c][:, 1023:1024]
                        nc.vector.tensor_tensor_scan(
                            tr, psA, sbB, init, op0=OP.min, op1=OP.min)
                        prev[c] = tr
                    nc.vector.tensor_tensor(
                        mins[:, i: i + 1], prev[0][:, 1023:1024],
                        prev[1][:, 1023:1024], op=OP.min)

            # ---------------- masked sum + count ---------------------------
            mask = keep.tile([128, iblocks], f32)
            nc.vector.tensor_scalar(mask, mins, 2.0, None, op0=OP.is_le)
            masked = keep.tile([128, iblocks], f32)
            nc.vector.tensor_mul(masked, mins, mask)
            acc = keep.tile([128, 2], f32)
            nc.vector.tensor_reduce(out=acc[:, 0:1], in_=masked, axis=AX.X, op=OP.add)
            nc.vector.tensor_reduce(out=acc[:, 1:2], in_=mask, axis=AX.X, op=OP.add)
            if spmd:
                # all-reduce the per-core sums so every core holds the global
                # result; host then fetches from a single device
                accd = nc.dram_tensor("accd", [128, 2], f32)
                accr = nc.dram_tensor("accr", [128, 2], f32)
                nc.sync.dma_start(out=accd[:, :], in_=acc)
                nc.gpsimd.collective_compute(
                    kind="AllReduce",
                    op=OP.add,
                    replica_groups=[list(range(N_CORES))],
                    ins=[accd[:, :]],
                    outs=[accr[:, :]],
                )
                nc.sync.dma_start(out=out[:, :], in_=accr[:, :])
            else:
                nc.sync.dma_start(out=out[:, :], in_=acc)

    nc.finalize()
    return nc


def _get_runner():
    if "runner" in _STATE:
        return _STATE["runner"]

    import jax
    from jax.sharding import Mesh, PartitionSpec
    from jax.experimental.shard_map import shard_map
    from concourse import mybir
    from concourse.bass2jax import (
        _bass_exec_p, install_neuronx_cc_hook, partition_id_tensor)

    nc = build_nc(spmd=True)
    install_neuronx_cc_hook()

    partition_name = (
        nc.partition_id_tensor.name if nc.partition_id_tensor else None)
    in_names, out_names, out_avals = [], [], []
    for alloc in nc.m.functions[0].allocations:
        if not isinstance(alloc, mybir.MemoryLocationSet):
            continue
        name = alloc.memorylocations[0].name
        if alloc.kind == "ExternalInput":
            if name != partition_name:
                in_names.append(name)
        elif alloc.kind == "ExternalOutput":
            shape = tuple(alloc.tensor_shape)
            dtype = mybir.dt.np(alloc.dtype)
            out_names.append(name)
            out_avals.append(jax.core.ShapedArray(shape, dtype))
    n_params, n_outs = len(in_names), len(out_avals)
    # outputs are fully written by the kernel, so no zero-donated output
    # operands are needed -- the custom call allocates them in shared_hbm
    in_names_all = in_names + ([partition_name] if partition_name else [])

    def _body(*args):
        operands = list(args)
        if partition_name is not None:
            operands.append(partition_id_tensor())
        return tuple(_bass_exec_p.bind(
            *operands,
            out_avals=tuple(out_avals),
            in_names=tuple(in_names_all),
            out_names=tuple(out_names),
            lowering_input_output_aliases=(),
            sim_require_finite=True,
            sim_require_nnan=True,
            nc=nc,
        ))

    mesh = Mesh(np.asarray(jax.devices()[:N_CORES]), ("core",))
    # out is identical on every core after the on-device AllReduce, so
    # declare it replicated -- the host fetches from one device only
    sharded = jax.jit(
        shard_map(
            _body, mesh=mesh,
            in_specs=(PartitionSpec("core"),) * n_params,
            out_specs=(PartitionSpec(),) * n_outs,
            check_rep=False),
        keep_unused=True)

    def run(pts):
        outs = sharded(pts)
        return np.asarray(outs[0])

    _STATE["runner"] = run
    return run


def kernel(pc0, pc1):
    nsh = NP // N_CORES
    # per-core rows: [pc0 shard (NQ); pc1 shard (nsh)] -> one global array;
    # assignment into the preallocated f16 buffer casts in one pass
    buf = _STATE.get("pts_buf")
    if buf is None:
        buf = _STATE["pts_buf"] = np.empty((N_CORES, NQ + nsh, 3), np.float16)
    buf[:, :NQ] = np.asarray(pc0).reshape(N_CORES, NQ, 3)
    buf[:, NQ:] = np.asarray(pc1).reshape(N_CORES, nsh, 3)
    run = _get_runner()
    if not _STATE.get("warm"):
        # the first post-compile execution pays a one-time NEFF/stream
        # warmup (~70ms); absorb it into the untimed first call
        _STATE["warm"] = True
        try:
            run(buf.reshape(N_CORES * (NQ + nsh), 3))
        except Exception:
            pass
    o = run(buf.reshape(N_CORES * (NQ + nsh), 3))  # replicated [128, 2]
    s = float(o[:, 0].sum(dtype=np.float64))
    cnt = float(o[:, 1].sum(dtype=np.float64))
    return np.array(s / cnt, dtype=np.float32)
